# revision 2
# baseline (speedup 1.0000x reference)
"""Multi-head attention (B=4, S=2048, D=512, H=8, Dh=64) on 8 trn2 NeuronCores.

Core c = b*2 + hg handles batch b, head-group hg (4 heads).

Host prep: X^T in bf16 (per batch), W slices in bf16, and W'_h = Wv_h @ Wo_h
folded on host (out-projection commutes with the per-(q,head) softmax
normalization), so the device does:
  Q^T = Wq^T X^T, K^T = Wk^T X^T          (bf16 stationary/moving, fp32 psum)
  V'_h = Xv W'_h  (keys on partitions)    + ones column
  per head, per 1024-q half, per 128-key block:
     scores^T slab [128k x 1024q] (PE) -> exp slab (ACT, the bottleneck)
  AV in natural orientation: U'[128q x 65] += P^T-block^T @ V'  (65-col moving
  carries the ones column so row-sums come for free), accumulated over the 16
  key blocks in PSUM, then DVE: reciprocal + (U * 1/r) summed across heads.
Out = [2048, 64] fp32 per core; host sums the two head-group partials and
adds bv @ Wo + bo.

ACT does nothing but the 128 exp instructions (the roofline: 131072
free-elements at 0.83 ns/el + per-instr bubbles ~= 133 us).
"""
import numpy as np

import concourse.bass as bass
import concourse.mybir as mybir
import concourse.tile as tile

F32 = mybir.dt.float32
BF16 = mybir.dt.bfloat16
NPBF16 = mybir.dt.np(BF16)

B, S, D_IN, H, D_HEAD = 4, 2048, 512, 8, 64
HG = 2
H_LOC = H // HG             # 4 heads per core
DO = H_LOC * D_HEAD         # 256 projected dims per core
N_CORES = B * HG
P = 128
KC = D_IN // P              # 4 contraction chunks
KB = S // P                 # 16 key blocks
QH = 2                      # q halves
QHS = S // QH               # 1024

# ---------------------------------------------------------------------------
# walrus in this container rejects >1 sync-wait per instruction: split the
# extras onto single-wait NOPs inserted before the instruction (same engine).
_ENGINES_WITH_NOP = {
    mybir.EngineType.PE,
    mybir.EngineType.Activation,
    mybir.EngineType.DVE,
    mybir.EngineType.Pool,
    mybir.EngineType.SP,
}


def _split_multi_waits(nc, max_waits=1):
    cnt = 0
    for fn in nc.m.functions:
        for blk in fn.blocks:
            out = []
            changed = False
            for inst in blk.instructions:
                si = getattr(inst, "sync_info", None)
                waits = list(si.on_wait) if si is not None else []
                if len(waits) > max_waits and inst.engine in _ENGINES_WITH_NOP:
                    changed = True
                    for w in waits[:-max_waits]:
                        cnt += 1
                        out.append(
                            mybir.InstNoOp(
                                name=f"I-wsplit-{cnt}",
                                engine=inst.engine,
                                ins=[],
                                outs=[],
                                sync_info=mybir.SyncInfo(on_wait=[w], on_update=[]),
                            )
                        )
                    inst.sync_info = mybir.SyncInfo(
                        on_wait=waits[-max_waits:], on_update=list(si.on_update)
                    )
                out.append(inst)
            if changed:
                blk.instructions = out


# ---------------------------------------------------------------------------


def build_program(loop_iters=None):
    nc = bass.Bass()

    xqt = nc.declare_dram_parameter("xqt", [D_IN, S], BF16, isOutput=False)
    xkt = nc.declare_dram_parameter("xkt", [D_IN, S], BF16, isOutput=False)
    xvt = nc.declare_dram_parameter("xvt", [D_IN, S], BF16, isOutput=False)
    wq = nc.declare_dram_parameter("wq", [D_IN, DO], BF16, isOutput=False)
    wk = nc.declare_dram_parameter("wk", [D_IN, DO], BF16, isOutput=False)
    wpr = nc.declare_dram_parameter("wpr", [D_IN, DO], BF16, isOutput=False)
    bqp = nc.declare_dram_parameter("bq", [DO], BF16, isOutput=False)  # ×0.125
    # unnormalized U^T (rows 0..63) + softmax row-sums (row 64) per
    # (head, q-half) slot; the host divides and transposes.
    out = nc.declare_dram_parameter(
        "out", [H_LOC * QH, D_HEAD + 1, QHS], F32, isOutput=True
    )

    with tile.TileContext(nc) as tc:
        with (
            tc.tile_pool(name="cst", bufs=1) as cst,
            tc.tile_pool(name="expool", bufs=8) as expool,
            tc.tile_pool(name="osbp", bufs=2) as osbp,
            tc.tile_pool(name="scp", bufs=2, space="PSUM") as scp,
            tc.tile_pool(name="otp", bufs=1, space="PSUM") as otp,
            tc.tile_pool(name="pp", bufs=2, space="PSUM") as pp,
        ):
            # ------------------------------------------------ persistent SBUF
            xq_sb = cst.tile([P, KC, S], BF16, name="xq_sb")
            xk_sb = cst.tile([P, KC, S], BF16, name="xk_sb")
            xv_sb = cst.tile([P, KC, S], BF16, name="xv_sb")
            wq_sb = cst.tile([P, KC, DO], BF16, name="wq_sb")
            wk_sb = cst.tile([P, KC, DO], BF16, name="wk_sb")
            wpr_sb = cst.tile([P, KC, DO], BF16, name="wpr_sb")
            bq_sb = cst.tile([P, HG, 1], BF16, name="bq_sb")
            qt = cst.tile([P, HG, S], BF16, name="qt")      # Q^T pairs
            kt = cst.tile([P, HG, S], BF16, name="kt")      # K^T pairs
            vpr = cst.tile([P, H_LOC, KB, D_HEAD + 1], BF16, name="vpr")
            csb = cst.tile([P, H_LOC, KB], F32, name="csb")  # per-key logit bias

            from contextlib import ExitStack as _ES
            _loop = _ES()
            if loop_iters is not None:
                _loop.enter_context(tc.For_i(0, loop_iters, 1))

            # ------------------------------------------------------ DMA loads
            # One serial DMA resource; issue order == criticality.
            xqv = xqt.rearrange("(c p) s -> p c s", p=P)
            xkv = xkt.rearrange("(c p) s -> p c s", p=P)
            xvv = xvt.rearrange("(c p) s -> p c s", p=P)
            nc.sync.dma_start(wq_sb[:], wq.rearrange("(c p) o -> p c o", p=P))
            nc.sync.dma_start(wk_sb[:], wk.rearrange("(c p) o -> p c o", p=P))
            nc.sync.dma_start(xk_sb[:, :, 0:128], xkv[:, :, 0:128])
            for pr in range(HG):
                nc.sync.dma_start(
                    bq_sb[:, pr, :], bqp[pr * P : (pr + 1) * P, None]
                )
            for kc in range(KC):
                nc.sync.dma_start(xq_sb[:, kc, 0:1024], xqv[:, kc, 0:1024])
            for kc in range(KC):
                nc.sync.dma_start(xk_sb[:, kc, 128:512], xkv[:, kc, 128:512])
            nc.sync.dma_start(wpr_sb[:], wpr.rearrange("(c p) o -> p c o", p=P))
            nc.sync.dma_start(xv_sb[:, :, 0:512], xvv[:, :, 0:512])
            nc.sync.dma_start(xv_sb[:, :, 512:1024], xvv[:, :, 512:1024])
            nc.sync.dma_start(xk_sb[:, :, 512:1024], xkv[:, :, 512:1024])
            nc.sync.dma_start(xk_sb[:, :, 1024:1536], xkv[:, :, 1024:1536])
            nc.sync.dma_start(xk_sb[:, :, 1536:2048], xkv[:, :, 1536:2048])
            nc.sync.dma_start(xq_sb[:, :, 1024:2048], xqv[:, :, 1024:2048])
            nc.sync.dma_start(xv_sb[:, :, 1024:1536], xvv[:, :, 1024:1536])
            nc.sync.dma_start(xv_sb[:, :, 1536:2048], xvv[:, :, 1536:2048])

            nc.gpsimd.memset(vpr[:, :, :, D_HEAD : D_HEAD + 1], 1.0)

            # ----------------------------------------------------- work items
            def proj_item(w_sb, x_sb, dst, pair, qc, lo=0):
                """Project one [128, 512] chunk of Q^T/K^T for `pair`."""
                def run():
                    n = 512 - lo
                    ps = pp.tile([P, 512], F32, tag="pp", name=f"pj{pair}q{qc}")
                    for kc in range(KC):
                        nc.tensor.matmul(
                            ps[:, 0:n],
                            w_sb[:, kc, pair * P : (pair + 1) * P],
                            x_sb[:, kc, qc * 512 + lo : (qc + 1) * 512],
                            start=(kc == 0),
                            stop=(kc == KC - 1),
                        )
                    nc.vector.tensor_copy(
                        out=dst[:, pair, qc * 512 + lo : (qc + 1) * 512],
                        in_=ps[:, 0:n],
                    )
                return run

            def c_item(h, qc, j0=0):
                """Per-key logit bias c = (0.125*bq_h) . K_h for keys qc*512.."""
                pair, hh = divmod(h, 2)
                rows = slice(hh * 64, hh * 64 + 64)

                def run():
                    ps = pp.tile([P, 512], F32, tag="pp", name=f"c{h}q{qc}")
                    for j in range(j0, 4):
                        kb = qc * 4 + j
                        nc.tensor.matmul(
                            ps[:, j : j + 1],
                            kt[rows, pair, kb * P : (kb + 1) * P],
                            bq_sb[rows, pair, :],
                            start=(j == j0),
                            stop=(j == 3),
                            skip_group_check=True,
                        )
                    nc.vector.tensor_copy(
                        out=csb[:, h, qc * 4 + j0 : (qc + 1) * 4],
                        in_=ps[:, j0:4],
                    )
                return run

            def vprime_item(kb):
                """V' for one key block, all 4 heads at once (one stationary
                per kc chunk serving a 256-wide moving W' block)."""
                def run():
                    ps = pp.tile([P, 512], F32, tag="pp", name=f"vp{kb}")
                    for kc in range(KC):
                        nc.tensor.matmul(
                            ps[:, 0:DO],
                            xv_sb[:, kc, kb * P : (kb + 1) * P],
                            wpr_sb[:, kc, :],
                            start=(kc == 0),
                            stop=(kc == KC - 1),
                        )
                    nc.vector.tensor_copy(
                        out=vpr[:, :, kb, 0:D_HEAD],
                        in_=ps[:, 0:DO].rearrange("p (h d) -> p h d", h=H_LOC),
                    )
                return run

            # --------------------------------------------------- filler lists
            fillers = {
                (0, 0): ([vprime_item(0), vprime_item(1),
                          proj_item(wk_sb, xk_sb, kt, 0, 1), c_item(0, 1),
                          vprime_item(2), vprime_item(3),
                          vprime_item(4), vprime_item(5),
                          proj_item(wk_sb, xk_sb, kt, 0, 2), c_item(0, 2),
                          vprime_item(6), vprime_item(7),
                          vprime_item(8), vprime_item(9),
                          proj_item(wk_sb, xk_sb, kt, 0, 3), c_item(0, 3)]
                         + [vprime_item(kb) for kb in range(10, 16)]
                         + [proj_item(wq_sb, xq_sb, qt, 0, 2),
                            proj_item(wq_sb, xq_sb, qt, 0, 3)]),
                (0, 1): [c_item(1, g) for g in range(4)],
                (1, 0): [proj_item(wk_sb, xk_sb, kt, 1, 0), c_item(2, 0),
                         proj_item(wk_sb, xk_sb, kt, 1, 1), c_item(2, 1),
                         proj_item(wq_sb, xq_sb, qt, 1, 0),
                         proj_item(wq_sb, xq_sb, qt, 1, 1)],
                (1, 1): [proj_item(wk_sb, xk_sb, kt, 1, 2), c_item(2, 2),
                         proj_item(wk_sb, xk_sb, kt, 1, 3), c_item(2, 3),
                         proj_item(wq_sb, xq_sb, qt, 1, 2),
                         proj_item(wq_sb, xq_sb, qt, 1, 3)],
                (2, 0): [c_item(3, g) for g in range(4)],
                (2, 1): [],
                (3, 0): [],
                (3, 1): [],
            }

            # ------------------------------------------------------- prologue
            proj_item(wq_sb, xq_sb, qt, 0, 0)()
            proj_item(wq_sb, xq_sb, qt, 0, 1)()
            # fast path for key block 0 so the exp stream starts early
            ps0 = pp.tile([P, 512], F32, tag="pp", name="kb0proj")
            for kc in range(KC):
                nc.tensor.matmul(
                    ps0[:, 0:P],
                    wk_sb[:, kc, 0:P],
                    xk_sb[:, kc, 0:P],
                    start=(kc == 0),
                    stop=(kc == KC - 1),
                )
            nc.vector.tensor_copy(out=kt[:, 0, 0:P], in_=ps0[:, 0:P])
            cps0 = pp.tile([P, 512], F32, tag="pp", name="kb0c")
            nc.tensor.matmul(cps0[:, 0:1], kt[0:64, 0, 0:P], bq_sb[0:64, 0, :],
                             start=True, stop=True)
            nc.vector.tensor_copy(out=csb[:, 0, 0:1], in_=cps0[:, 0:1])

            # ------------------------------------------------- attention loop
            early = {(0, 0): [proj_item(wk_sb, xk_sb, kt, 0, 0, lo=128),
                              c_item(0, 0, j0=1)]}
            ex_tiles = {}

            def ot_step_for(ot, h_, half_, kb):
                ex_t = ex_tiles[(h_, half_, kb)]
                for j in range(2):
                    nc.tensor.matmul(
                        ot[:, j, :],
                        vpr[:, h_, kb, :],
                        ex_t[:, j, :],
                        start=(kb == 0),
                        stop=(kb == KB - 1),
                    )

            LAG = 3
            carry = []
            for h in range(H_LOC):
                pair, hh = divmod(h, 2)
                rows = slice(hh * 64, hh * 64 + 64)
                for half in range(QH):
                    slot = h * QH + half
                    items = fillers[(h, half)]
                    n_items = len(items)
                    done = 0
                    ot_holder = []

                    def ot_step(kb):
                        if kb == 0:
                            ot_holder.append(
                                otp.tile([D_HEAD + 1, 2, 512], F32, tag="ot",
                                         name=f"ot{slot}")
                            )
                        ot_step_for(ot_holder[-1], h, half, kb)

                    for kb in range(KB):
                        slab = scp.tile([P, 2, 512], F32, tag="sc",
                                        name=f"sc{h}h{half}k{kb}")
                        for j in range(2):
                            nc.tensor.matmul(
                                slab[:, j, :],
                                kt[rows, pair, kb * P : (kb + 1) * P],
                                qt[rows, pair,
                                   half * QHS + j * 512 : half * QHS + (j + 1) * 512],
                                start=True,
                                stop=True,
                            )
                        ex_t = expool.tile([P, 2, 512], BF16, tag="ex",
                                           name=f"ex{h}h{half}k{kb}")
                        ex_tiles[(h, half, kb)] = ex_t
                        nc.scalar.activation(
                            ex_t[:], slab[:],
                            mybir.ActivationFunctionType.Exp,
                            bias=csb[:, h, kb : kb + 1],
                            scale=float(1.0 / np.sqrt(D_HEAD)),
                        )
                        if kb == 0:
                            for fn in early.get((h, half), []):
                                fn()
                        if kb == 1 and carry:
                            for fn in carry:
                                fn()
                            carry = []
                        if kb >= 2:
                            want = min(n_items, kb * n_items // 13)
                            while done < want:
                                items[done]()
                                done += 1
                        if kb >= LAG:
                            ot_step(kb - LAG)
                    while done < n_items:
                        items[done]()
                        done += 1

                    def _finish(slot_, ot_, h_, half_, split=False):
                        def fin():
                            for r in range(KB - LAG, KB):
                                ot_step_for(ot_, h_, half_, r)
                            osb = osbp.tile([D_HEAD + 1, 2, 512], F32,
                                            tag="osb", name=f"osb{slot_}")
                            if split:
                                for j in range(2):
                                    nc.vector.tensor_copy(
                                        out=osb[:, j, :], in_=ot_[:, j, :]
                                    )
                                    nc.scalar.dma_start(
                                        out[slot_, :, j * 512 : (j + 1) * 512],
                                        osb[:, j, :],
                                    )
                            else:
                                nc.vector.tensor_copy(out=osb[:], in_=ot_[:])
                                nc.scalar.dma_start(
                                    out[slot_, :, :],
                                    osb[:].rearrange("p a b -> p (a b)"),
                                )
                        return fin

                    if (h, half) == (H_LOC - 1, QH - 1):
                        _finish(slot, ot_holder[-1], h, half, split=True)()
                    else:
                        carry = [_finish(slot, ot_holder[-1], h, half)]

            _loop.close()

    _split_multi_waits(nc)
    return nc


class _Runner:
    """Compile once; keep a jitted shard_map executable around."""

    def __init__(self, nc=None):
        import jax
        from jax.experimental.shard_map import shard_map
        from jax.sharding import Mesh, NamedSharding, PartitionSpec
        from concourse import bass2jax

        bass2jax.install_neuronx_cc_hook()
        if nc is None:
            nc = build_program()
        self.nc = nc
        self.jax = jax

        partition_name = (
            nc.partition_id_tensor.name if nc.partition_id_tensor else None
        )
        in_names, out_names, out_avals, zero_outs = [], [], [], []
        for alloc in nc.m.functions[0].allocations:
            if not isinstance(alloc, mybir.MemoryLocationSet):
                continue
            name = alloc.memorylocations[0].name
            if alloc.kind == "ExternalInput":
                if name != partition_name:
                    in_names.append(name)
            elif alloc.kind == "ExternalOutput":
                out_names.append(name)
                shape = tuple(alloc.tensor_shape)
                dtype = mybir.dt.np(alloc.dtype)
                out_avals.append(jax.core.ShapedArray(shape, dtype))
                zero_outs.append(np.zeros(shape, dtype))
        self.in_names = list(in_names)
        self.out_names = out_names
        self.out_avals = out_avals
        self.zero_outs = zero_outs
        n_params = len(in_names)
        n_outs = len(out_avals)
        all_in_names = in_names + out_names
        if partition_name is not None:
            all_in_names.append(partition_name)
        donate = tuple(range(n_params, n_params + n_outs))

        def _body(*args):
            operands = list(args)
            if partition_name is not None:
                operands.append(bass2jax.partition_id_tensor())
            outs = bass2jax._bass_exec_p.bind(
                *operands,
                out_avals=tuple(out_avals),
                in_names=tuple(all_in_names),
                out_names=tuple(out_names),
                lowering_input_output_aliases=(),
                sim_require_finite=True,
                sim_require_nnan=True,
                nc=nc,
            )
            return tuple(outs)

        devices = jax.devices()[:N_CORES]
        mesh = Mesh(np.asarray(devices), ("core",))
        self.mesh = mesh
        self.sharding = NamedSharding(mesh, PartitionSpec("core"))
        in_specs = (PartitionSpec("core"),) * (n_params + n_outs)
        out_specs = (PartitionSpec("core"),) * len(out_names)
        self.fn = jax.jit(
            shard_map(
                _body, mesh=mesh, in_specs=in_specs,
                out_specs=out_specs, check_rep=False,
            ),
            donate_argnums=donate,
            keep_unused=True,
        )

    def put_inputs(self, in_maps):
        concat = [
            np.concatenate([np.asarray(in_maps[c][n]) for c in range(N_CORES)], axis=0)
            for n in self.in_names
        ]
        return [self.jax.device_put(a, self.sharding) for a in concat]

    def make_zeros(self):
        return [
            self.jax.device_put(
                np.zeros((N_CORES * z.shape[0], *z.shape[1:]), z.dtype), self.sharding
            )
            for z in self.zero_outs
        ]

    def run(self, in_dev):
        out_arrs = self.fn(*in_dev, *self.make_zeros())
        return [
            {
                n: np.asarray(out_arrs[i]).reshape(N_CORES, *self.out_avals[i].shape)[c]
                for i, n in enumerate(self.out_names)
            }
            for c in range(N_CORES)
        ]


_RUNNER = None


def _get_runner():
    global _RUNNER
    if _RUNNER is None:
        _RUNNER = _Runner()
    return _RUNNER


def _make_in_maps(query, key, value, Wq, Wk, Wv, Wo, bq, bk=None):
    xts = {}
    for b in range(B):
        xts[b] = (
            np.ascontiguousarray(query[b].T).astype(NPBF16),
            np.ascontiguousarray(key[b].T).astype(NPBF16),
            np.ascontiguousarray(value[b].T).astype(NPBF16),
        )
    wslices = []
    for hg in range(HG):
        sl = slice(hg * DO, (hg + 1) * DO)
        wq_s = np.ascontiguousarray(Wq[:, sl]).astype(NPBF16)
        wk_s = np.ascontiguousarray(Wk[:, sl]).astype(NPBF16)
        wpr_s = np.empty((D_IN, DO), dtype=np.float32)
        for h in range(H_LOC):
            g = slice((hg * H_LOC + h) * D_HEAD, (hg * H_LOC + h + 1) * D_HEAD)
            wpr_s[:, h * D_HEAD : (h + 1) * D_HEAD] = Wv[:, g] @ Wo[g, :]
        wslices.append((wq_s, wk_s, wpr_s.astype(NPBF16),
                        np.ascontiguousarray(bq[sl] * 0.125).astype(NPBF16)))
    in_maps = []
    for c in range(N_CORES):
        b, hg = divmod(c, HG)
        wq_s, wk_s, wpr_s, bq_s = wslices[hg]
        in_maps.append(
            {
                "xqt": xts[b][0],
                "xkt": xts[b][1],
                "xvt": xts[b][2],
                "wq": wq_s,
                "wk": wk_s,
                "wpr": wpr_s,
                "bq": bq_s,
            }
        )
    return in_maps


def kernel(query, key, value, Wq, bq, Wk, bk, Wv, bv, Wo, bo):
    query = np.ascontiguousarray(np.asarray(query, dtype=np.float32))
    key = np.ascontiguousarray(np.asarray(key, dtype=np.float32))
    value = np.ascontiguousarray(np.asarray(value, dtype=np.float32))
    Wq = np.asarray(Wq, dtype=np.float32)
    Wk = np.asarray(Wk, dtype=np.float32)
    Wv = np.asarray(Wv, dtype=np.float32)
    Wo = np.asarray(Wo, dtype=np.float32)
    bq = np.asarray(bq, dtype=np.float32)
    bk = np.asarray(bk, dtype=np.float32)
    bv = np.asarray(bv, dtype=np.float32)
    bo = np.asarray(bo, dtype=np.float32)

    r = _get_runner()
    in_dev = r.put_inputs(_make_in_maps(query, key, value, Wq, Wk, Wv, Wo, bq))
    results = r.run(in_dev)

    out = np.zeros((B, S, D_HEAD), dtype=np.float32)
    for c in range(N_CORES):
        b = c // HG
        oc = results[c]["out"]  # [H_LOC*QH, 65, QHS]
        for h in range(H_LOC):
            for half in range(QH):
                sl = oc[h * QH + half]
                out[b, half * QHS : (half + 1) * QHS, :] += (
                    sl[0:D_HEAD, :] / sl[D_HEAD, :][None, :]
                ).T
    out += bv @ Wo + bo
    return out


def bench(query, key, value, Wq, bq, Wk, bk, Wv, bv, Wo, bo, iters=20):
    """Steady-state per-iteration wall time of the device execution."""
    import time

    r = _get_runner()
    in_dev = r.put_inputs(
        _make_in_maps(
            np.asarray(query, np.float32), np.asarray(key, np.float32),
            np.asarray(value, np.float32), np.asarray(Wq, np.float32),
            np.asarray(Wk, np.float32), np.asarray(Wv, np.float32),
            np.asarray(Wo, np.float32), np.asarray(bq, np.float32),
            np.asarray(bk, np.float32),
        )
    )
    outs = r.fn(*in_dev, *r.make_zeros())
    self_jax = r.jax
    self_jax.block_until_ready(outs)
    zeros = [r.make_zeros() for _ in range(iters)]
    t0 = time.monotonic()
    last = None
    for i in range(iters):
        last = r.fn(*in_dev, *zeros[i])
    self_jax.block_until_ready(last)
    t1 = time.monotonic()
    return (t1 - t0) / iters


# revision 3
# speedup vs baseline: 1.1793x; 1.1793x over previous
"""Multi-head attention (B=4, S=2048, D=512, H=8, Dh=64) on 8 trn2 NeuronCores.

Core c = b*2 + hg handles batch b, head-group hg (4 heads).

Host prep: X^T in bf16 (per batch), W slices in bf16, and W'_h = Wv_h @ Wo_h
folded on host (out-projection commutes with the per-(q,head) softmax
normalization), so the device does:
  Q^T = Wq^T X^T, K^T = Wk^T X^T          (bf16 stationary/moving, fp32 psum)
  V'_h = Xv W'_h  (keys on partitions)    + ones column
  per head, per 1024-q half, per 128-key block:
     scores^T slab [128k x 1024q] (PE) -> exp slab (ACT, the bottleneck)
  AV in natural orientation: U'[128q x 65] += P^T-block^T @ V'  (65-col moving
  carries the ones column so row-sums come for free), accumulated over the 16
  key blocks in PSUM, then DVE: reciprocal + (U * 1/r) summed across heads.
Out = [2048, 64] fp32 per core; host sums the two head-group partials and
adds bv @ Wo + bo.

ACT does nothing but the 128 exp instructions (the roofline: 131072
free-elements at 0.83 ns/el + per-instr bubbles ~= 133 us).
"""
import numpy as np

import concourse.bass as bass
import concourse.mybir as mybir
import concourse.tile as tile

F32 = mybir.dt.float32
BF16 = mybir.dt.bfloat16
NPBF16 = mybir.dt.np(BF16)

B, S, D_IN, H, D_HEAD = 4, 2048, 512, 8, 64
HG = 2
H_LOC = H // HG             # 4 heads per core
DO = H_LOC * D_HEAD         # 256 projected dims per core
N_CORES = B * HG
P = 128
KC = D_IN // P              # 4 contraction chunks
KB = S // P                 # 16 key blocks
QH = 2                      # q halves
QHS = S // QH               # 1024

# ---------------------------------------------------------------------------
# walrus in this container rejects >1 sync-wait per instruction: split the
# extras onto single-wait NOPs inserted before the instruction (same engine).
_ENGINES_WITH_NOP = {
    mybir.EngineType.PE,
    mybir.EngineType.Activation,
    mybir.EngineType.DVE,
    mybir.EngineType.Pool,
    mybir.EngineType.SP,
}


def _split_multi_waits(nc, max_waits=1):
    cnt = 0
    for fn in nc.m.functions:
        for blk in fn.blocks:
            out = []
            changed = False
            for inst in blk.instructions:
                si = getattr(inst, "sync_info", None)
                waits = list(si.on_wait) if si is not None else []
                if len(waits) > max_waits and inst.engine in _ENGINES_WITH_NOP:
                    changed = True
                    for w in waits[:-max_waits]:
                        cnt += 1
                        out.append(
                            mybir.InstNoOp(
                                name=f"I-wsplit-{cnt}",
                                engine=inst.engine,
                                ins=[],
                                outs=[],
                                sync_info=mybir.SyncInfo(on_wait=[w], on_update=[]),
                            )
                        )
                    inst.sync_info = mybir.SyncInfo(
                        on_wait=waits[-max_waits:], on_update=list(si.on_update)
                    )
                out.append(inst)
            if changed:
                blk.instructions = out


# ---------------------------------------------------------------------------


def build_program(loop_iters=None):
    nc = bass.Bass()

    xqt = nc.declare_dram_parameter("xqt", [D_IN, S], BF16, isOutput=False)
    xkt = nc.declare_dram_parameter("xkt", [D_IN, S], BF16, isOutput=False)
    xvt = nc.declare_dram_parameter("xvt", [D_IN, S], BF16, isOutput=False)
    wq = nc.declare_dram_parameter("wq", [D_IN, DO], BF16, isOutput=False)
    wk = nc.declare_dram_parameter("wk", [D_IN, DO], BF16, isOutput=False)
    wpr = nc.declare_dram_parameter("wpr", [D_IN, DO], BF16, isOutput=False)
    bqp = nc.declare_dram_parameter("bq", [DO], BF16, isOutput=False)  # ×0.125
    # unnormalized U^T (rows 0..63) + softmax row-sums (row 64) per
    # (head, q-half) slot; the host divides and transposes.
    out = nc.declare_dram_parameter(
        "out", [H_LOC * QH, D_HEAD + 1, QHS], F32, isOutput=True
    )

    with tile.TileContext(nc) as tc:
        with (
            tc.tile_pool(name="cst", bufs=1) as cst,
            tc.tile_pool(name="expool", bufs=8) as expool,
            tc.tile_pool(name="osbp", bufs=2) as osbp,
            tc.tile_pool(name="scp", bufs=2, space="PSUM") as scp,
            tc.tile_pool(name="otp", bufs=1, space="PSUM") as otp,
            tc.tile_pool(name="pp", bufs=2, space="PSUM") as pp,
        ):
            # ------------------------------------------------ persistent SBUF
            xq_sb = cst.tile([P, KC, S], BF16, name="xq_sb")
            xk_sb = cst.tile([P, KC, S], BF16, name="xk_sb")
            xv_sb = cst.tile([P, KC, S], BF16, name="xv_sb")
            wq_sb = cst.tile([P, KC, DO], BF16, name="wq_sb")
            wk_sb = cst.tile([P, KC, DO], BF16, name="wk_sb")
            wpr_sb = cst.tile([P, KC, DO], BF16, name="wpr_sb")
            bq_sb = cst.tile([P, HG, 1], BF16, name="bq_sb")
            qt = cst.tile([P, HG, S], BF16, name="qt")      # Q^T pairs
            kt = cst.tile([P, HG, S], BF16, name="kt")      # K^T pairs
            vpr = cst.tile([P, H_LOC, KB, D_HEAD + 1], BF16, name="vpr")
            csb = cst.tile([P, H_LOC, KB], F32, name="csb")  # per-key logit bias

            from contextlib import ExitStack as _ES
            _loop = _ES()
            if loop_iters is not None:
                _loop.enter_context(tc.For_i(0, loop_iters, 1))

            # ------------------------------------------------------ DMA loads
            # One serial DMA resource; issue order == criticality.
            xqv = xqt.rearrange("(c p) s -> p c s", p=P)
            xkv = xkt.rearrange("(c p) s -> p c s", p=P)
            xvv = xvt.rearrange("(c p) s -> p c s", p=P)
            nc.sync.dma_start(wq_sb[:], wq.rearrange("(c p) o -> p c o", p=P))
            for kc in range(KC):
                nc.sync.dma_start(xq_sb[:, kc, 0:1024], xqv[:, kc, 0:1024])
            nc.sync.dma_start(wk_sb[:], wk.rearrange("(c p) o -> p c o", p=P))
            for pr in range(HG):
                nc.sync.dma_start(
                    bq_sb[:, pr, :], bqp[pr * P : (pr + 1) * P, None]
                )
            for kc in range(KC):
                nc.sync.dma_start(xk_sb[:, kc, 0:512], xkv[:, kc, 0:512])
            nc.sync.dma_start(wpr_sb[:], wpr.rearrange("(c p) o -> p c o", p=P))
            nc.sync.dma_start(xv_sb[:, :, 0:512], xvv[:, :, 0:512])
            nc.sync.dma_start(xv_sb[:, :, 512:1024], xvv[:, :, 512:1024])
            nc.sync.dma_start(xk_sb[:, :, 512:1024], xkv[:, :, 512:1024])
            nc.sync.dma_start(xk_sb[:, :, 1024:1536], xkv[:, :, 1024:1536])
            nc.sync.dma_start(xv_sb[:, :, 1024:1536], xvv[:, :, 1024:1536])
            nc.sync.dma_start(xk_sb[:, :, 1536:2048], xkv[:, :, 1536:2048])
            nc.sync.dma_start(xv_sb[:, :, 1536:2048], xvv[:, :, 1536:2048])
            nc.sync.dma_start(xq_sb[:, :, 1024:2048], xqv[:, :, 1024:2048])

            nc.gpsimd.memset(vpr[:, :, :, D_HEAD : D_HEAD + 1], 1.0)

            # ----------------------------------------------------- work items
            def proj_item(w_sb, x_sb, dst, pair, qc, lo=0):
                """Project one [128, 512] chunk of Q^T/K^T for `pair`."""
                def run():
                    n = 512 - lo
                    ps = pp.tile([P, 512], F32, tag="pp", name=f"pj{pair}q{qc}")
                    for kc in range(KC):
                        nc.tensor.matmul(
                            ps[:, 0:n],
                            w_sb[:, kc, pair * P : (pair + 1) * P],
                            x_sb[:, kc, qc * 512 + lo : (qc + 1) * 512],
                            start=(kc == 0),
                            stop=(kc == KC - 1),
                        )
                    nc.vector.tensor_copy(
                        out=dst[:, pair, qc * 512 + lo : (qc + 1) * 512],
                        in_=ps[:, 0:n],
                    )
                return run

            def c_item(h, qc, j0=0):
                """Per-key logit bias c = (0.125*bq_h) . K_h for keys qc*512.."""
                pair, hh = divmod(h, 2)
                rows = slice(hh * 64, hh * 64 + 64)

                def run():
                    ps = pp.tile([P, 512], F32, tag="pp", name=f"c{h}q{qc}")
                    for j in range(j0, 4):
                        kb = qc * 4 + j
                        nc.tensor.matmul(
                            ps[:, j : j + 1],
                            kt[rows, pair, kb * P : (kb + 1) * P],
                            bq_sb[rows, pair, :],
                            start=(j == j0),
                            stop=(j == 3),
                            skip_group_check=True,
                        )
                    nc.vector.tensor_copy(
                        out=csb[:, h, qc * 4 + j0 : (qc + 1) * 4],
                        in_=ps[:, j0:4],
                    )
                return run

            def vprime_item(kb):
                """V' for one key block, all 4 heads at once (one stationary
                per kc chunk serving a 256-wide moving W' block)."""
                def run():
                    ps = pp.tile([P, 512], F32, tag="pp", name=f"vp{kb}")
                    for kc in range(KC):
                        nc.tensor.matmul(
                            ps[:, 0:DO],
                            xv_sb[:, kc, kb * P : (kb + 1) * P],
                            wpr_sb[:, kc, :],
                            start=(kc == 0),
                            stop=(kc == KC - 1),
                        )
                    nc.vector.tensor_copy(
                        out=vpr[:, :, kb, 0:D_HEAD],
                        in_=ps[:, 0:DO].rearrange("p (h d) -> p h d", h=H_LOC),
                    )
                return run

            # --------------------------------------------------- filler lists
            fillers = {
                (0, 0): ([vprime_item(0), vprime_item(1),
                          proj_item(wk_sb, xk_sb, kt, 0, 1), c_item(0, 1),
                          vprime_item(2), vprime_item(3),
                          vprime_item(4), vprime_item(5),
                          proj_item(wk_sb, xk_sb, kt, 0, 2), c_item(0, 2),
                          vprime_item(6), vprime_item(7),
                          vprime_item(8), vprime_item(9),
                          proj_item(wk_sb, xk_sb, kt, 0, 3), c_item(0, 3)]
                         + [vprime_item(kb) for kb in range(10, 16)]
                         + [proj_item(wq_sb, xq_sb, qt, 0, 2),
                            proj_item(wq_sb, xq_sb, qt, 0, 3)]),
                (0, 1): [c_item(1, g) for g in range(4)],
                (1, 0): [proj_item(wk_sb, xk_sb, kt, 1, 0), c_item(2, 0),
                         proj_item(wk_sb, xk_sb, kt, 1, 1), c_item(2, 1),
                         proj_item(wq_sb, xq_sb, qt, 1, 0),
                         proj_item(wq_sb, xq_sb, qt, 1, 1)],
                (1, 1): [proj_item(wk_sb, xk_sb, kt, 1, 2), c_item(2, 2),
                         proj_item(wk_sb, xk_sb, kt, 1, 3), c_item(2, 3),
                         proj_item(wq_sb, xq_sb, qt, 1, 2),
                         proj_item(wq_sb, xq_sb, qt, 1, 3)],
                (2, 0): [c_item(3, g) for g in range(4)],
                (2, 1): [],
                (3, 0): [],
                (3, 1): [],
            }

            # ------------------------------------------------------- prologue
            proj_item(wq_sb, xq_sb, qt, 0, 0)()
            proj_item(wq_sb, xq_sb, qt, 0, 1)()
            proj_item(wk_sb, xk_sb, kt, 0, 0)()
            c_item(0, 0)()

            # ------------------------------------------------- attention loop
            ex_tiles = {}

            def ot_step_for(ot, h_, half_, kb):
                ex_t = ex_tiles[(h_, half_, kb)]
                for j in range(2):
                    nc.tensor.matmul(
                        ot[:, j, :],
                        vpr[:, h_, kb, :],
                        ex_t[:, j, :],
                        start=(kb == 0),
                        stop=(kb == KB - 1),
                    )

            LAG = 3
            carry = []
            for h in range(H_LOC):
                pair, hh = divmod(h, 2)
                rows = slice(hh * 64, hh * 64 + 64)
                for half in range(QH):
                    slot = h * QH + half
                    items = fillers[(h, half)]
                    n_items = len(items)
                    done = 0
                    ot_holder = []

                    def ot_step(kb):
                        if kb == 0:
                            ot_holder.append(
                                otp.tile([D_HEAD + 1, 2, 512], F32, tag="ot",
                                         name=f"ot{slot}")
                            )
                        ot_step_for(ot_holder[-1], h, half, kb)

                    for kb in range(KB):
                        slab = scp.tile([P, 2, 512], F32, tag="sc",
                                        name=f"sc{h}h{half}k{kb}")
                        for j in range(2):
                            nc.tensor.matmul(
                                slab[:, j, :],
                                kt[rows, pair, kb * P : (kb + 1) * P],
                                qt[rows, pair,
                                   half * QHS + j * 512 : half * QHS + (j + 1) * 512],
                                start=True,
                                stop=True,
                            )
                        ex_t = expool.tile([P, 2, 512], BF16, tag="ex",
                                           name=f"ex{h}h{half}k{kb}")
                        ex_tiles[(h, half, kb)] = ex_t
                        nc.scalar.activation(
                            ex_t[:], slab[:],
                            mybir.ActivationFunctionType.Exp,
                            bias=csb[:, h, kb : kb + 1],
                            scale=float(1.0 / np.sqrt(D_HEAD)),
                        )
                        if kb == 1 and carry:
                            for fn in carry:
                                fn()
                            carry = []
                        if kb >= 2:
                            want = min(n_items, kb * n_items // 13)
                            while done < want:
                                items[done]()
                                done += 1
                        if kb >= LAG:
                            ot_step(kb - LAG)
                    while done < n_items:
                        items[done]()
                        done += 1

                    def _finish(slot_, ot_, h_, half_, split=False):
                        def fin():
                            for r in range(KB - LAG, KB):
                                ot_step_for(ot_, h_, half_, r)
                            osb = osbp.tile([D_HEAD + 1, 2, 512], F32,
                                            tag="osb", name=f"osb{slot_}")
                            if split:
                                for j in range(2):
                                    nc.vector.tensor_copy(
                                        out=osb[:, j, :], in_=ot_[:, j, :]
                                    )
                                    nc.scalar.dma_start(
                                        out[slot_, :, j * 512 : (j + 1) * 512],
                                        osb[:, j, :],
                                    )
                            else:
                                nc.vector.tensor_copy(out=osb[:], in_=ot_[:])
                                nc.scalar.dma_start(
                                    out[slot_, :, :],
                                    osb[:].rearrange("p a b -> p (a b)"),
                                )
                        return fin

                    if (h, half) == (H_LOC - 1, QH - 1):
                        _finish(slot, ot_holder[-1], h, half)()
                    else:
                        carry = [_finish(slot, ot_holder[-1], h, half)]

            _loop.close()

    _split_multi_waits(nc)
    return nc


class _Runner:
    """Compile once; keep a jitted shard_map executable around."""

    def __init__(self, nc=None):
        import jax
        from jax.experimental.shard_map import shard_map
        from jax.sharding import Mesh, NamedSharding, PartitionSpec
        from concourse import bass2jax

        bass2jax.install_neuronx_cc_hook()
        if nc is None:
            nc = build_program()
        self.nc = nc
        self.jax = jax

        partition_name = (
            nc.partition_id_tensor.name if nc.partition_id_tensor else None
        )
        in_names, out_names, out_avals, zero_outs = [], [], [], []
        for alloc in nc.m.functions[0].allocations:
            if not isinstance(alloc, mybir.MemoryLocationSet):
                continue
            name = alloc.memorylocations[0].name
            if alloc.kind == "ExternalInput":
                if name != partition_name:
                    in_names.append(name)
            elif alloc.kind == "ExternalOutput":
                out_names.append(name)
                shape = tuple(alloc.tensor_shape)
                dtype = mybir.dt.np(alloc.dtype)
                out_avals.append(jax.core.ShapedArray(shape, dtype))
                zero_outs.append(np.zeros(shape, dtype))
        self.in_names = list(in_names)
        self.out_names = out_names
        self.out_avals = out_avals
        self.zero_outs = zero_outs
        n_params = len(in_names)
        n_outs = len(out_avals)
        all_in_names = in_names + out_names
        if partition_name is not None:
            all_in_names.append(partition_name)
        donate = tuple(range(n_params, n_params + n_outs))

        def _body(*args):
            operands = list(args)
            if partition_name is not None:
                operands.append(bass2jax.partition_id_tensor())
            outs = bass2jax._bass_exec_p.bind(
                *operands,
                out_avals=tuple(out_avals),
                in_names=tuple(all_in_names),
                out_names=tuple(out_names),
                lowering_input_output_aliases=(),
                sim_require_finite=True,
                sim_require_nnan=True,
                nc=nc,
            )
            return tuple(outs)

        devices = jax.devices()[:N_CORES]
        mesh = Mesh(np.asarray(devices), ("core",))
        self.mesh = mesh
        self.sharding = NamedSharding(mesh, PartitionSpec("core"))
        in_specs = (PartitionSpec("core"),) * (n_params + n_outs)
        out_specs = (PartitionSpec("core"),) * len(out_names)
        self.fn = jax.jit(
            shard_map(
                _body, mesh=mesh, in_specs=in_specs,
                out_specs=out_specs, check_rep=False,
            ),
            donate_argnums=donate,
            keep_unused=True,
        )

    def put_inputs(self, in_maps):
        concat = [
            np.concatenate([np.asarray(in_maps[c][n]) for c in range(N_CORES)], axis=0)
            for n in self.in_names
        ]
        return [self.jax.device_put(a, self.sharding) for a in concat]

    def make_zeros(self):
        return [
            self.jax.device_put(
                np.zeros((N_CORES * z.shape[0], *z.shape[1:]), z.dtype), self.sharding
            )
            for z in self.zero_outs
        ]

    def run(self, in_dev):
        out_arrs = self.fn(*in_dev, *self.make_zeros())
        return [
            {
                n: np.asarray(out_arrs[i]).reshape(N_CORES, *self.out_avals[i].shape)[c]
                for i, n in enumerate(self.out_names)
            }
            for c in range(N_CORES)
        ]


_RUNNER = None


def _get_runner():
    global _RUNNER
    if _RUNNER is None:
        _RUNNER = _Runner()
    return _RUNNER


def _make_in_maps(query, key, value, Wq, Wk, Wv, Wo, bq, bk=None):
    xts = {}
    for b in range(B):
        xts[b] = (
            np.ascontiguousarray(query[b].T).astype(NPBF16),
            np.ascontiguousarray(key[b].T).astype(NPBF16),
            np.ascontiguousarray(value[b].T).astype(NPBF16),
        )
    wslices = []
    for hg in range(HG):
        sl = slice(hg * DO, (hg + 1) * DO)
        wq_s = np.ascontiguousarray(Wq[:, sl]).astype(NPBF16)
        wk_s = np.ascontiguousarray(Wk[:, sl]).astype(NPBF16)
        wpr_s = np.empty((D_IN, DO), dtype=np.float32)
        for h in range(H_LOC):
            g = slice((hg * H_LOC + h) * D_HEAD, (hg * H_LOC + h + 1) * D_HEAD)
            wpr_s[:, h * D_HEAD : (h + 1) * D_HEAD] = Wv[:, g] @ Wo[g, :]
        wslices.append((wq_s, wk_s, wpr_s.astype(NPBF16),
                        np.ascontiguousarray(bq[sl] * 0.125).astype(NPBF16)))
    in_maps = []
    for c in range(N_CORES):
        b, hg = divmod(c, HG)
        wq_s, wk_s, wpr_s, bq_s = wslices[hg]
        in_maps.append(
            {
                "xqt": xts[b][0],
                "xkt": xts[b][1],
                "xvt": xts[b][2],
                "wq": wq_s,
                "wk": wk_s,
                "wpr": wpr_s,
                "bq": bq_s,
            }
        )
    return in_maps


def kernel(query, key, value, Wq, bq, Wk, bk, Wv, bv, Wo, bo):
    query = np.ascontiguousarray(np.asarray(query, dtype=np.float32))
    key = np.ascontiguousarray(np.asarray(key, dtype=np.float32))
    value = np.ascontiguousarray(np.asarray(value, dtype=np.float32))
    Wq = np.asarray(Wq, dtype=np.float32)
    Wk = np.asarray(Wk, dtype=np.float32)
    Wv = np.asarray(Wv, dtype=np.float32)
    Wo = np.asarray(Wo, dtype=np.float32)
    bq = np.asarray(bq, dtype=np.float32)
    bk = np.asarray(bk, dtype=np.float32)
    bv = np.asarray(bv, dtype=np.float32)
    bo = np.asarray(bo, dtype=np.float32)

    r = _get_runner()
    in_dev = r.put_inputs(_make_in_maps(query, key, value, Wq, Wk, Wv, Wo, bq))
    results = r.run(in_dev)

    out = np.zeros((B, S, D_HEAD), dtype=np.float32)
    for c in range(N_CORES):
        b = c // HG
        oc = results[c]["out"]  # [H_LOC*QH, 65, QHS]
        for h in range(H_LOC):
            for half in range(QH):
                sl = oc[h * QH + half]
                out[b, half * QHS : (half + 1) * QHS, :] += (
                    sl[0:D_HEAD, :] / sl[D_HEAD, :][None, :]
                ).T
    out += bv @ Wo + bo
    return out


def bench(query, key, value, Wq, bq, Wk, bk, Wv, bv, Wo, bo, iters=20):
    """Steady-state per-iteration wall time of the device execution."""
    import time

    r = _get_runner()
    in_dev = r.put_inputs(
        _make_in_maps(
            np.asarray(query, np.float32), np.asarray(key, np.float32),
            np.asarray(value, np.float32), np.asarray(Wq, np.float32),
            np.asarray(Wk, np.float32), np.asarray(Wv, np.float32),
            np.asarray(Wo, np.float32), np.asarray(bq, np.float32),
            np.asarray(bk, np.float32),
        )
    )
    outs = r.fn(*in_dev, *r.make_zeros())
    self_jax = r.jax
    self_jax.block_until_ready(outs)
    zeros = [r.make_zeros() for _ in range(iters)]
    t0 = time.monotonic()
    last = None
    for i in range(iters):
        last = r.fn(*in_dev, *zeros[i])
    self_jax.block_until_ready(last)
    t1 = time.monotonic()
    return (t1 - t0) / iters
